# revision 1
# baseline (speedup 1.0000x reference)
"""Trainium2 Bass kernel for B-spline curve evaluation (nn_CurveEval).

Problem: cubic B-spline evaluation. For each of B=8192 curves with M=100
control points (DIM=3) and a clamped knot vector of K=104 knots, evaluate
the curve at T=512 fixed uniform parameter values.

Strategy (pure data parallel, batch sharded 8 ways):
  Per core: 1024 curves, processed in 8 tiles of 128 curves (partitions).
  1. Span search in knot space: for each interior knot j compute the first
     eval index c_j with u[c] - knot_j > 1e-8 (analytic guess + exact
     verification, no per-t work).
  2. All span-dependent per-t quantities (6 shifted knot values, 12 control
     point values) are nondecreasing staircase functions of t.  Materialize
     each by scattering its raw f32 bits (as an int16 pair) at the span
     boundaries with gpsimd local_scatter, then a fill-forward select-scan
     (state = mask*state + value) on the vector engine.  This is exact (no
     rounding) and avoids any per-(b,t) gather, which this HW lacks.
  3. Cox-de Boor recursion + 4-tap control-point combine as dense
     elementwise f32 ops on [128, 512] tiles.
Curves with near-duplicate knots (any gap < 5e-7 within distance 3, where
the reference's own numerics become discontinuous) are recomputed exactly
on host -- expected ~0.1% of curves.
"""

import os
from contextlib import ExitStack

import numpy as np

import concourse.bass as bass
import concourse.mybir as mybir
import concourse.tile as tile
from concourse import library_config
from concourse.bass_utils import run_bass_kernel_spmd

# ---------------------------------------------------------------- constants
B, M, PDEG, DIM, T, K = 8192, 100, 3, 3, 512, 104
NCORES = 8
BL = B // NCORES      # curves per core
PT = 128              # curves per tile (partition dim)
NT = BL // PT         # tiles per core
NI = 96               # interior knots per curve

U0 = np.float32(1e-5)
UEND = np.float32(1.0 - 1e-5)
STEP = np.float32((UEND - U0) / np.float32(511.0))
EPS8 = np.float32(1e-8)
MAGIC = np.float32(12582912.0)   # 1.5*2^23: float round-to-int magic

F32 = mybir.dt.float32
I16 = mybir.dt.int16

AOP = mybir.AluOpType


# XLA-CPU's constant-folded linspace loop (from the optimized HLO):
#   step = t*C1 ; om = 1-step ; u = fma(start, om, t*C2) ; u[511] = stop
LS_C1 = np.float32(1.0) / np.float32(511.0)            # 0.00195694715f
LS_C2 = np.float32(UEND * (np.float32(1.0) / np.float32(511.0)))


def _u_grid() -> np.ndarray:
    # bitwise replica of jnp.linspace(1e-5, 1-1e-5, 512, float32) as
    # compiled by XLA CPU (verified bit-exact against the jitted fusion).
    t = np.arange(T, dtype=np.float32)
    step = (t * LS_C1).astype(np.float32)
    om = (np.float32(1.0) - step).astype(np.float32)
    u = np.float32(
        np.float64(U0) * np.float64(om) + np.float64(t) * np.float64(LS_C2)
    ).astype(np.float32)
    u[511] = UEND
    return u


# ------------------------------------------------------------- bass program
def _build_nc() -> bass.Bass:
    nc = bass.Bass()
    ctrl = nc.declare_dram_parameter("ctrl", [BL, M * DIM], F32, isOutput=False)
    knots = nc.declare_dram_parameter("knots", [BL, K], F32, isOutput=False)
    uin = nc.declare_dram_parameter("u", [PT, T], F32, isOutput=False)
    out = nc.declare_dram_parameter("out", [BL, T * DIM], F32, isOutput=True)

    with tile.TileContext(nc) as tc, ExitStack() as ctx:
        singles = ctx.enter_context(tc.tile_pool(name="singles", bufs=1))
        io = ctx.enter_context(tc.tile_pool(name="io", bufs=NT))
        small = ctx.enter_context(tc.tile_pool(name="small", bufs=2))
        dsts = ctx.enter_context(tc.tile_pool(name="dsts", bufs=19))
        stairs = ctx.enter_context(tc.tile_pool(name="stairs", bufs=18))
        work = ctx.enter_context(tc.tile_pool(name="work", bufs=1))
        outp = ctx.enter_context(tc.tile_pool(name="outp", bufs=NT))

        nc.gpsimd.load_library(library_config.local_scatter)
        u_t = singles.tile([PT, T], F32)
        nc.sync.dma_start(out=u_t[:, :], in_=uin[:, :])
        ones16 = singles.tile([PT, NI], I16)
        nc.vector.memset(ones16[:, :], 1)
        neg1 = singles.tile([PT, NI], F32)
        nc.vector.memset(neg1[:, :], -1.0)

        for it in range(NT):
            r0 = it * PT
            Ud = io.tile([PT, K], F32, tag="Ud")
            nc.sync.dma_start(out=Ud[:, :], in_=knots[r0 : r0 + PT, :])
            # scatter operands must come from a single producer proc (the
            # generic ISA encoding has a 1-wait budget), so bounce the
            # DMA-landed inputs through DVE copies.
            U = io.tile([PT, K], F32, tag="U")
            nc.vector.tensor_copy(U[:, :], Ud[:, :])
            Pd = []
            for d in range(DIM):
                pdd = io.tile([PT, M], F32, tag=f"Pd{d}", name=f"pdd{d}_{it}")
                nc.sync.dma_start(
                    out=pdd[:, :], in_=ctrl[r0 : r0 + PT, d :: DIM]
                )
                pd = io.tile([PT, M], F32, tag=f"P{d}", name=f"pd{d}_{it}")
                nc.vector.tensor_copy(pd[:, :], pdd[:, :])
                Pd.append(pd)

            # ---------------- stage A: span boundaries c_j  [PT, NI] -----
            intr = U[:, 4 : 4 + NI]
            q = small.tile([PT, NI], F32, tag="q")
            # q = (knot + (1e-8 - 1e-5)) / step
            nc.vector.tensor_scalar(
                q[:, :], intr,
                float(EPS8) - float(U0), float(1.0 / np.float64(STEP)),
                AOP.add, AOP.mult,
            )
            c0 = small.tile([PT, NI], F32, tag="c0")
            nc.vector.tensor_scalar(
                c0[:, :], q[:, :], float(MAGIC), float(-MAGIC), AOP.add, AOP.add
            )
            # count qualifies(c0 + delta) for delta in -2..1
            acc = small.tile([PT, NI], F32, tag="acc")
            tauu = small.tile([PT, NI], F32, tag="tauu")
            stt = small.tile([PT, NI], F32, tag="stt")
            om = small.tile([PT, NI], F32, tag="om")
            ge = small.tile([PT, NI], F32, tag="ge")
            for i, dlt in enumerate((-2.0, -1.0, 0.0, 1.0)):
                # tau = c0+dlt; u_tau replicates the XLA linspace loop
                # (sequential f32 rounding; the fused-FMA 1-ulp deviation
                # only matters within C2-continuity of the spline: harmless)
                nc.vector.tensor_scalar(
                    tauu[:, :], c0[:, :], dlt, None, AOP.add
                )
                nc.vector.tensor_scalar(
                    stt[:, :], tauu[:, :], float(LS_C1), None, AOP.mult
                )
                nc.vector.tensor_scalar(
                    om[:, :], stt[:, :], -float(U0), float(U0),
                    AOP.mult, AOP.add,
                )  # om = U0*(1 - step) = -U0*step + U0  (see note below)
                nc.vector.tensor_scalar(
                    stt[:, :], tauu[:, :], float(LS_C2), None, AOP.mult
                )
                nc.vector.tensor_tensor(tauu[:, :], stt[:, :], om[:, :], AOP.add)
                nc.vector.tensor_tensor(tauu[:, :], tauu[:, :], intr, AOP.subtract)
                dst_g = acc if i == 0 else ge
                nc.vector.tensor_scalar(
                    dst_g[:, :], tauu[:, :], float(EPS8), None, AOP.is_gt
                )
                if i > 0:
                    nc.vector.tensor_tensor(acc[:, :], acc[:, :], ge[:, :], AOP.add)
            # c = clamp(c0 + 2 - acc, 0, 511)
            cc = small.tile([PT, NI], F32, tag="cc")
            nc.vector.tensor_scalar(cc[:, :], c0[:, :], 2.0, None, AOP.add)
            nc.vector.tensor_tensor(cc[:, :], cc[:, :], acc[:, :], AOP.subtract)
            nc.vector.tensor_scalar(
                cc[:, :], cc[:, :], 0.0, 511.0, AOP.max, AOP.min
            )
            # mask duplicates (same bin): keep last of each run
            eq = small.tile([PT, NI - 1], mybir.dt.uint8, tag="eq")
            nc.vector.tensor_tensor(
                eq[:, :], cc[:, : NI - 1], cc[:, 1:NI], AOP.is_equal
            )
            nc.vector.copy_predicated(cc[:, : NI - 1], eq[:, :], neg1[:, : NI - 1])
            # index tensors for the scatters
            idx1 = small.tile([PT, NI], I16, tag="idx1")
            nc.vector.tensor_copy(idx1[:, :], cc[:, :])
            c2 = small.tile([PT, NI], F32, tag="c2")
            nc.vector.tensor_scalar(c2[:, :], cc[:, :], 2.0, None, AOP.mult)
            idxp = small.tile([PT, 2 * NI], I16, tag="idxp")
            idxp_v = idxp[:, :].rearrange("p (a b) -> p a b", b=2)
            nc.vector.tensor_copy(idxp_v[:, :, 0], c2[:, :])
            nc.vector.tensor_scalar(c2[:, :], c2[:, :], 1.0, None, AOP.add)
            nc.vector.tensor_copy(idxp_v[:, :, 1], c2[:, :])

            # ---------------- stage B: staircases via scatter + scan -----
            flagd = dsts.tile([PT, T], I16, tag="flagd", bufs=2)
            nc.gpsimd.local_scatter(
                flagd[:, :], ones16[:, :], idx1[:, :],
                channels=PT, num_elems=T, num_idxs=NI,
            )
            m = work.tile([PT, T], F32, tag="m", bufs=2)
            nc.vector.tensor_scalar(
                m[:, :], flagd[:, :], -1.0, 1.0, AOP.mult, AOP.add
            )
            # fence: a cheap DVE op that reads m so the DVE self-wait for
            # tick(m) attaches HERE (plain tensor_tensor, multi-wait ok)
            # rather than on the scans (TensorScalarPtr has a 1-wait
            # budget; their only remaining wait is the Pool dst).
            fence = work.tile([PT, 2], F32, tag="fence")
            nc.vector.tensor_tensor(
                fence[:, :], m[:, 0:2], m[:, 0:2], AOP.add
            )

            def staircase(name, data96_f32, init_col, monotone):
                dst = dsts.tile([PT, 2 * T], I16, tag="dst")
                nc.gpsimd.local_scatter(
                    dst[:, :], data96_f32.bitcast(I16), idxp[:, :],
                    channels=PT, num_elems=2 * T, num_idxs=2 * NI,
                )
                st = stairs.tile([PT, T], F32, tag="stair")
                if monotone:
                    # knot staircases are nondecreasing and >= 0 with 0
                    # holes: fill-forward == running max of the raw dst
                    nc.vector.tensor_tensor_scan(
                        st[:, :], dst[:, :].bitcast(F32),
                        dst[:, :].bitcast(F32), init_col,
                        AOP.max, AOP.bypass,
                    )
                else:
                    nc.vector.tensor_tensor_scan(
                        st[:, :], m[:, :], dst[:, :].bitcast(F32), init_col,
                        AOP.mult, AOP.add,
                    )
                return st

            # 6 knot staircases: value U[s+o], boundary-j value U[(j+4)+o]
            SU = {}
            for o in (-2, -1, 0, 1, 2, 3):
                SU[o] = staircase(
                    f"u{o}", U[:, 4 + o : 4 + o + NI], U[:, 3 + o : 4 + o],
                    monotone=True,
                )
            # 12 ctrl staircases: value P_d[s-3+l], boundary value P_d[j+1+l]
            SP = {}
            for l in range(PDEG + 1):
                for d in range(DIM):
                    SP[(l, d)] = staircase(
                        f"p{l}{d}",
                        Pd[d][:, 1 + l : 1 + l + NI],
                        Pd[d][:, l : l + 1],
                        monotone=False,
                    )

            # ---------------- per-t math ---------------------------------
            def tt(eng, op, o, a, b):
                eng.tensor_tensor(o[:, :], a[:, :], b[:, :], op)

            # a_o = U[s+o] - u ; b_o = u - U[s+o]
            a1 = work.tile([PT, T], F32, tag="a1")
            a2 = work.tile([PT, T], F32, tag="a2")
            a3 = work.tile([PT, T], F32, tag="a3")
            b0 = work.tile([PT, T], F32, tag="b0")
            bm1 = work.tile([PT, T], F32, tag="bm1")
            bm2 = work.tile([PT, T], F32, tag="bm2")
            for ao, o in ((a1, 1), (a2, 2), (a3, 3)):
                nc.vector.affine_then_add(ao[:, :], u_t[:, :], SU[o][:, :], -1.0, 0.0)
            for bo, o in ((b0, 0), (bm1, -1), (bm2, -2)):
                nc.vector.affine_then_add(bo[:, :], SU[o][:, :], u_t[:, :], -1.0, 0.0)

            den = work.tile([PT, T], F32, tag="den")
            rscr = work.tile([PT, T], F32, tag="rscr")
            rec = []
            for i in range(3):
                rec_i = work.tile([PT, T], F32, tag=f"rec{i}", name=f"rec{i}_{it}")
                rec.append(rec_i)

            def recip(dst, aa, bb):
                tt(nc.vector, AOP.add, den, aa, bb)
                nc.vector.reciprocal_approx_accurate(
                    dst[:, :], den[:, :], scratch=rscr[:, :]
                )

            N0 = work.tile([PT, T], F32, tag="N0")
            N1 = work.tile([PT, T], F32, tag="N1")
            N2 = work.tile([PT, T], F32, tag="N2")
            N3 = work.tile([PT, T], F32, tag="N3")
            sv = work.tile([PT, T], F32, tag="sv")
            sv2 = work.tile([PT, T], F32, tag="sv2")
            tmp = work.tile([PT, T], F32, tag="tmp")
            tmp2 = work.tile([PT, T], F32, tag="tmp2")

            # k=1
            recip(rec[0], a1, b0)
            tt(nc.vector, AOP.mult, N0, a1, rec[0])     # N0 = a1*r
            tt(nc.vector, AOP.mult, sv, b0, rec[0])     # N1 = sv = b0*r
            # k=2, r=0  (denom a1+bm1)
            recip(rec[1], a1, bm1)
            tt(nc.vector, AOP.mult, tmp, N0, rec[1])
            tt(nc.vector, AOP.mult, N0, a1, tmp)
            tt(nc.vector, AOP.mult, sv2, bm1, tmp)
            # k=2, r=1  (denom a2+b0)
            recip(rec[2], a2, b0)
            tt(nc.vector, AOP.mult, tmp, sv, rec[2])
            tt(nc.vector, AOP.mult, tmp2, a2, tmp)
            tt(nc.vector, AOP.add, N1, sv2, tmp2)
            tt(nc.vector, AOP.mult, sv, b0, tmp)        # N2 seed
            # k=3, r=0  (denom a1+bm2)
            recip(rec[0], a1, bm2)
            tt(nc.vector, AOP.mult, tmp, N0, rec[0])
            tt(nc.vector, AOP.mult, N0, a1, tmp)
            tt(nc.vector, AOP.mult, sv2, bm2, tmp)
            # k=3, r=1  (denom a2+bm1)
            recip(rec[1], a2, bm1)
            tt(nc.vector, AOP.mult, tmp, N1, rec[1])
            tt(nc.vector, AOP.mult, tmp2, a2, tmp)
            tt(nc.vector, AOP.add, N1, sv2, tmp2)
            tt(nc.vector, AOP.mult, sv2, bm1, tmp)
            # k=3, r=2  (denom a3+b0)
            recip(rec[2], a3, b0)
            tt(nc.vector, AOP.mult, tmp, sv, rec[2])
            tt(nc.vector, AOP.mult, tmp2, a3, tmp)
            tt(nc.vector, AOP.add, N2, sv2, tmp2)
            tt(nc.vector, AOP.mult, N3, b0, tmp)

            # ---------------- combine: out[t,d] = sum_l N_l * P_{l,d} ----
            ob = outp.tile([PT, T * DIM], F32, tag="ob")
            obv = ob[:, :].rearrange("p (t d) -> p t d", d=DIM)
            Ns = (N0, N1, N2, N3)
            for d in range(DIM):
                ov = obv[:, :, d]
                nc.vector.tensor_tensor(ov, N0[:, :], SP[(0, d)][:, :], AOP.mult)
                for l in range(1, PDEG + 1):
                    nc.vector.tensor_tensor(
                        tmp[:, :], Ns[l][:, :], SP[(l, d)][:, :], AOP.mult
                    )
                    nc.vector.tensor_tensor(ov, ov, tmp[:, :], AOP.add)

            nc.sync.dma_start(out=out[r0 : r0 + PT, :], in_=ob[:, :])

    # populate .instr bytes for extended-inst InstISA subclasses
    # (local_scatter); raw Bass skips this Bacc pass and the NEFF
    # compiler rejects empty payloads with "ISA wrong length".
    from concourse.library_overlay import lower_extended_insts

    lower_extended_insts(nc)

    # LocalScatter's generic ISA encoding accepts one sync wait; Tile puts
    # two on slot-reusing scatters: Pool>=k (slot's previous writer, WAW)
    # and DVE>=v (operand producers + the slot's previous readers).  The
    # Pool wait is transitively implied: each previous reader (a DVE
    # instruction included in v) itself waited Pool>=k before reading.
    import bass_rust as _br

    for inst in nc.all_instructions():
        tn = type(inst).__name__
        si = inst.sync_info
        if not si or len(si.on_wait) <= 1:
            continue
        if tn == "InstLocalScatter":
            keep = [w for w in si.on_wait if "Pool" not in w.ant_name]
            assert len(keep) == 1 and "DVE" in keep[0].ant_name, si.on_wait
            inst.sync_info = _br.SyncInfo(on_wait=keep, on_update=si.on_update)
        elif tn == "InstDMACopy":
            # out-DMA carries {DVE (ob producer), DMAHW_q (an input DMA
            # whose completion the DVE chain already waited on)}; the
            # direct-2D DMA encoding has a 1-wait budget.
            keep = [w for w in si.on_wait if "DMAHW" not in w.ant_name]
            if len(keep) == 1:
                inst.sync_info = _br.SyncInfo(
                    on_wait=keep, on_update=si.on_update
                )

    # Kernel-tail drain aggregates 10 waits (8 DMA queues + DVE + Pool) --
    # far over the Drain encoding's budget.  Only the queues whose LAST
    # DMA is an output write need waiting on (input-queue completions and
    # the DVE/Pool ticks are transitively implied by the out-DMAs' own
    # waits).  Keep one such wait on the drain and spread the rest across
    # the zero-wait barrier-protocol instructions that follow it.
    insts = list(nc.all_instructions())
    big_i = None
    for i, inst in enumerate(insts):
        si = inst.sync_info
        if type(inst).__name__ == "InstDrain" and si and len(si.on_wait) > 2:
            big_i = i
            break
    if big_i is not None:
        last_q = {}
        for inst in insts[:big_i]:
            if type(inst).__name__ == "InstDMACopy" and inst.sync_info:
                is_out = any(
                    "out" in str(getattr(o, "memref", "")) for o in inst.outs
                )
                for u in inst.sync_info.on_update:
                    if "DMAHW" in u.ant_name:
                        last_q[u.ant_name] = is_out
        drain = insts[big_i]
        req = [
            w
            for w in drain.sync_info.on_wait
            if "DMAHW" in w.ant_name and last_q.get(w.ant_name, True)
        ]
        assert req, drain.sync_info.on_wait
        drain.sync_info = _br.SyncInfo(
            on_wait=req[:1], on_update=drain.sync_info.on_update
        )
        todo = req[1:]
        for inst in insts[big_i - 6 :]:
            if not todo:
                break
            if inst is drain:
                continue
            si = inst.sync_info
            if type(inst).__name__ in (
                "InstDrain",
                "InstEventSemaphore",
                "InstUnconditionalBranch",
            ) and (not si or not si.on_wait):
                inst.sync_info = _br.SyncInfo(
                    on_wait=[todo.pop(0)],
                    on_update=(si.on_update if si else []),
                )
        assert not todo, f"unplaced drain waits: {todo}"
    return nc


_NC_CACHE: list = [None]
TRACE = False
LAST_RESULTS: list = [None]


def _get_nc():
    if _NC_CACHE[0] is None:
        _NC_CACHE[0] = _build_nc()
    return _NC_CACHE[0]


# ------------------------------------------------------- host-side helpers
def _ref_numpy(ctrl_pts: np.ndarray, knot_u: np.ndarray) -> np.ndarray:
    """Exact f32 replica of the jax reference for a subset of curves."""
    n = ctrl_pts.shape[0]
    u = _u_grid()                                        # [T]
    Uk = knot_u
    diff = u[None, None, :] - Uk[:, PDEG:-PDEG, None]    # [n, M-P+1, T]
    masked = np.where(diff > EPS8, diff, np.float32(1.0))
    uspan = np.argmin(masked, axis=1).astype(np.int64) + PDEG   # [n, T]

    def gknots(off):
        return np.take_along_axis(Uk, uspan + off, axis=1)

    Ni = [None] * (PDEG + 1)
    Ni[0] = np.broadcast_to(np.ones_like(u), (n, T)).copy()
    for k in range(1, PDEG + 1):
        saved = np.zeros((n, T), np.float32)
        for r in range(k):
            U1 = gknots(r + 1)
            U2 = gknots(1 - k + r)
            denom = (U1 - u[None, :]) + (u[None, :] - U2)
            safe = np.where(denom == 0.0, np.float32(1.0), denom)
            temp = np.where(denom == 0.0, np.float32(1e-4), Ni[r] / safe)
            Ni[r] = saved + (U1 - u[None, :]) * temp
            saved = (u[None, :] - U2) * temp
        Ni[k] = saved
    Nu = np.stack(Ni, axis=1)                            # [n, P+1, T]
    idx = uspan[:, :, None] - PDEG + np.arange(PDEG + 1)  # [n, T, P+1]
    pts = np.take_along_axis(
        ctrl_pts[:, :, None, :], idx[:, :, :, None].transpose(0, 2, 1, 3), axis=1
    )  # -> [n, T?, ...]: do it simply instead
    pts = ctrl_pts[np.arange(n)[:, None, None], idx]     # [n, T, P+1, DIM]
    curve = np.einsum("blt,btld->btd", Nu, pts).astype(np.float32)
    return curve


def _flag_curves(knot_u: np.ndarray) -> np.ndarray:
    """Curves where some reference Cox-de-Boor denominator U[i+k]-U[i] is
    below 5e-7 (reference numerics discontinuous there, and the device's
    approximate reciprocal / span handling may diverge).  The denominator
    pairs (i, i+k) that actually occur are: k=1: i in [3,99];
    k=2: i in [2,99]; k=3: i in [1,99]."""
    bad = np.zeros(knot_u.shape[0], dtype=bool)
    for k, ilo in ((1, 3), (2, 2), (3, 1)):
        g = knot_u[:, ilo + k : 100 + k] - knot_u[:, ilo:100]
        bad |= (g < np.float32(5e-7)).any(axis=1)
    return bad


# ---------------------------------------------------------------- entry
def kernel(ctrl_pts: np.ndarray, knot_u: np.ndarray) -> np.ndarray:
    ctrl_pts = np.ascontiguousarray(ctrl_pts, dtype=np.float32)
    knot_u = np.ascontiguousarray(knot_u, dtype=np.float32)

    nc = _get_nc()
    u_rep = np.broadcast_to(_u_grid()[None, :], (PT, T)).copy()

    in_maps = []
    for c in range(NCORES):
        sl = slice(c * BL, (c + 1) * BL)
        in_maps.append(
            {
                "ctrl": ctrl_pts[sl].reshape(BL, M * DIM),
                "knots": knot_u[sl],
                "u": u_rep,
            }
        )
    res = run_bass_kernel_spmd(
        nc, in_maps, core_ids=list(range(NCORES)), trace=TRACE
    )
    LAST_RESULTS[0] = res
    out = np.concatenate(
        [res.results[c]["out"].reshape(BL, T, DIM) for c in range(NCORES)], axis=0
    )

    bad = _flag_curves(knot_u)
    if bad.any():
        out[bad] = _ref_numpy(ctrl_pts[bad], knot_u[bad])
    return out

